# revision 1
# baseline (speedup 1.0000x reference)
"""MoE layer (N=8192, D=1024, F=4096, E=8, top-2) on 8 Trainium2 NeuronCores.

Strategy (expert-parallel, matches the sharding hint):
  - Host: gate (inputs @ Wg + bg), top-k selection, softmax combine weights,
    and the dispatch/combine index plumbing (gather tokens per expert,
    scatter-add expert outputs back). This is the tiny O(N*D*E) part.
  - Device (SPMD, core e == expert e): the heavy FFN
        y = silu(x_e @ W1[e] + b1[e]) @ W2[e]  scaled per-row by the
    combine weight.

Per-core kernel layout:
  mm1: h^T[f, t] = W1[d, f]^T @ x^T[d, t]   (stationary = W1 tile, moving = x^T)
  silu+bias on ScalarE (PSUM -> SBUF), h^T kept resident in SBUF
  mm2: y[t, d]  = h^T[f, t]^T @ W2[f, d]    (stationary = h^T tile, moving = W2)
  scale rows by combine weight on VectorE, DMA out.
Tokens are processed in blocks of <=768 so mm2 can hold block/128 PSUM
accumulators per 512-wide half of D.

Two device variants:
  - "bf16_resident" (default): weights converted to bf16 on host and kept
    fully resident in SBUF (64+64 KB/partition); activations bf16; fp32
    PSUM accumulate.  Fast LDWEIGHTS (FWL), no weight re-streaming.
  - "f32r_stream": everything fp32r (full-rate fp32 matmul); weights are
    re-streamed per token block.  ~10x more accurate, somewhat slower.
"""

import os
import sys
import types

import numpy as np

import concourse.bass as bass
import concourse.bacc as bacc
import concourse.mybir as mybir
import concourse.tile as tile
from concourse.bass_utils import run_bass_kernel_spmd


def _ensure_ntff_hook():
    """Provide antenv.axon_hooks if the image lacks it, so trace=True (or a
    caller-set BASS_TRACE=1) degrades gracefully instead of crashing in
    run_bass_kernel_spmd. Uses the same ctypes NTFF hook the axon boot
    would install when available."""
    try:
        import antenv.axon_hooks  # noqa: F401

        return
    except ImportError:
        pass
    hook = None
    try:
        from trn_agent_boot.trn_boot import _ntff_profile_via_ctypes

        hook = _ntff_profile_via_ctypes("/opt/axon/libaxon_pjrt.so")
    except Exception:
        hook = None
    m = types.ModuleType("antenv.axon_hooks")
    m.get_axon_ntff_profile_hook = lambda: hook
    m.set_axon_ntff_profile_hook = lambda h: None
    sys.modules["antenv.axon_hooks"] = m
    try:
        import antenv

        antenv.axon_hooks = m
    except ImportError:
        pass


_ensure_ntff_hook()

F32 = mybir.dt.float32
F32R = mybir.dt.float32r
BF16 = mybir.dt.bfloat16

D_MODEL = 1024
D_FF = 4096
N_EXPERTS = 8
N_CORES = 8
MAX_BLK = 768  # tokens per block; block/128 PSUM banks used in mm2 per D-half

MODE = os.environ.get("MOE_KERNEL_MODE", "bf16_resident")

# exec time (ns) of the most recent device run, when tracing was enabled
LAST_EXEC_TIME_NS = None
_NC_CACHE = {}


def _split_blocks(C):
    """Split C (multiple of 128) into blocks of at most MAX_BLK tokens."""
    blocks = []
    t = C
    while t > 0:
        b = min(t, MAX_BLK)
        blocks.append(b)
        t -= b
    return blocks


def _split_subtiles(blk):
    """Split a block into moving-dim subtiles <=512 (one PSUM bank)."""
    out = []
    t = blk
    while t > 0:
        s = min(t, 512)
        out.append(s)
        t -= s
    return out


def _build_nc_bf16_resident(C):
    """bf16 weights fully resident in SBUF; bf16 activations; f32 psum.

    Host pre-shuffles all inputs to partition-major chunk layouts so every
    DMA is 128 fully-contiguous descriptors:
      w1: [8, 128, 8, 512]   (f-chunk, partition, d-chunk, f-within)
      w2: [4, 128, 8, 1024]  (f-chunk, partition, f-within, d)
      x:  [nb, 128, 8, 768]  (block, partition, d-chunk, token)
      b1: [128, 32]  cw: [128, C/128]
    """
    nc = bacc.Bacc("TRN2", target_bir_lowering=False, debug=False)
    D, F = D_MODEL, D_FF
    nf = F // 128  # 32
    nd = D // 128  # 8
    blocks = _split_blocks(C)
    nb = len(blocks)

    w1 = nc.declare_dram_parameter("w1", [8, 128, nd, F // 8], BF16, isOutput=False)
    w2 = nc.declare_dram_parameter("w2", [4, 128, nf // 4, D], BF16, isOutput=False)
    xT = nc.declare_dram_parameter("xT", [nb, 128, nd, MAX_BLK], BF16, isOutput=False)
    b1 = nc.declare_dram_parameter("b1", [128, nf], F32, isOutput=False)
    cw = nc.declare_dram_parameter("cw", [128, C // 128], F32, isOutput=False)
    y = nc.declare_dram_parameter("y", [C, D], F32, isOutput=True)

    with tile.TileContext(nc) as tc:
        with (
            tc.tile_pool(name="const", bufs=1) as constp,
            tc.tile_pool(name="wres", bufs=1) as wres,
            tc.tile_pool(name="xp", bufs=1) as xp,
            tc.tile_pool(name="hp", bufs=1) as hp,
            tc.tile_pool(name="yp", bufs=3) as yp,
            tc.tile_pool(name="ps1", bufs=2, space="PSUM") as ps1,
            tc.tile_pool(name="ps2", bufs=6, space="PSUM") as ps2,
        ):
            w1_sb = wres.tile([128, 8, nd, F // 8], BF16, tag="w1")
            w2_sb = wres.tile([128, 4, nf // 4, D], BF16, tag="w2")
            x_first = xp.tile([128, nd, MAX_BLK], BF16, tag="x")
            # first w1 chunk + first x block gate the first matmuls; issue
            # them (and the rest of w1) from sync (HWDGE).  w2/b1/cw go via
            # gpsimd: SWDGE is slow but those aren't needed for 40+ us.
            nc.sync.dma_start(w1_sb[:, 0], w1[0])
            nc.sync.dma_start(x_first[:], xT[0])
            for c in range(1, 8):
                nc.sync.dma_start(w1_sb[:, c], w1[c])
            b1_sb = constp.tile([128, nf], F32, tag="b1")
            nc.gpsimd.dma_start(b1_sb[:], b1[:])
            cw_sb = constp.tile([128, C // 128], F32, tag="cw")
            nc.gpsimd.dma_start(cw_sb[:], cw[:])
            for c in range(4):
                nc.gpsimd.dma_start(w2_sb[:, c], w2[c])

            t0 = 0
            for bi, blk in enumerate(blocks):
                ntt = blk // 128
                if bi == 0:
                    x_sb = x_first
                else:
                    x_sb = xp.tile([128, nd, MAX_BLK], BF16, tag="x")
                    nc.sync.dma_start(x_sb[:], xT[bi])
                h_sb = hp.tile([128, nf, MAX_BLK], BF16, tag="h")

                # ---- phase 1: h^T = silu(W1^T x^T + b1) ----
                for f in range(nf):
                    s0 = 0
                    for ts in _split_subtiles(blk):
                        ph = ps1.tile([128, 512], F32, tag="ph")
                        for d in range(nd):
                            nc.tensor.matmul(
                                ph[:, :ts],
                                w1_sb[:, f // 4, d, (f % 4) * 128 : (f % 4 + 1) * 128],
                                x_sb[:, d, s0 : s0 + ts],
                                start=(d == 0),
                                stop=(d == nd - 1),
                            )
                        nc.scalar.activation(
                            h_sb[:, f, s0 : s0 + ts],
                            ph[:, :ts],
                            mybir.ActivationFunctionType.Silu,
                            bias=b1_sb[:, f : f + 1],
                        )
                        s0 += ts

                # ---- phase 2: y = (h^T)^T W2, scaled by combine weight ----
                for dh in range(2):
                    pys = [
                        ps2.tile([128, 512], F32, tag="py", name=f"py{i}")
                        for i in range(ntt)
                    ]
                    for f in range(nf):
                        for tt in range(ntt):
                            nc.tensor.matmul(
                                pys[tt][:],
                                h_sb[:, f, tt * 128 : (tt + 1) * 128],
                                w2_sb[:, f // 8, f % 8, dh * 512 : (dh + 1) * 512],
                                start=(f == 0),
                                stop=(f == nf - 1),
                            )
                    for tt in range(ntt):
                        g = t0 // 128 + tt
                        y_sb = yp.tile([128, 512], F32, tag="y")
                        nc.vector.tensor_scalar_mul(
                            y_sb[:], pys[tt][:], cw_sb[:, g : g + 1]
                        )
                        nc.sync.dma_start(
                            y[t0 + tt * 128 : t0 + (tt + 1) * 128,
                              dh * 512 : (dh + 1) * 512],
                            y_sb[:],
                        )
                t0 += blk
    nc.finalize()  # Bacc: runs wait-legalization + register allocation
    return nc


def _build_nc_f32r_stream(C):
    """All-fp32r variant; weights re-streamed per token block.

    Host layouts (partition-major, fully contiguous DMAs):
      w1: [32, 128, 8, 128]  (f-tile, partition, d-chunk, f-within)
      w2: [32, 2, 128, 512]  (f-tile, d-half, partition, d-within)
      x:  [nb, 128, 8, 768]  b1: [128, 32]  cw: [128, C/128]
    """
    nc = bacc.Bacc("TRN2", target_bir_lowering=False, debug=False)
    D, F = D_MODEL, D_FF
    nf = F // 128
    nd = D // 128
    blocks = _split_blocks(C)
    nb = len(blocks)

    w1 = nc.declare_dram_parameter("w1", [nf, 128, nd, 128], F32R, isOutput=False)
    w2 = nc.declare_dram_parameter("w2", [nf, 2, 128, 512], F32R, isOutput=False)
    xT = nc.declare_dram_parameter("xT", [nb, 128, nd, MAX_BLK], F32R, isOutput=False)
    b1 = nc.declare_dram_parameter("b1", [128, nf], F32, isOutput=False)
    cw = nc.declare_dram_parameter("cw", [128, C // 128], F32, isOutput=False)
    y = nc.declare_dram_parameter("y", [C, D], F32, isOutput=True)

    with tile.TileContext(nc) as tc:
        with (
            tc.tile_pool(name="const", bufs=1) as constp,
            tc.tile_pool(name="xp", bufs=2) as xp,
            tc.tile_pool(name="hp", bufs=1) as hp,
            tc.tile_pool(name="w1p", bufs=4) as w1p,
            tc.tile_pool(name="w2p", bufs=8) as w2p,
            tc.tile_pool(name="yp", bufs=3) as yp,
            tc.tile_pool(name="ps1", bufs=2, space="PSUM") as ps1,
            tc.tile_pool(name="ps2", bufs=6, space="PSUM") as ps2,
        ):
            b1_sb = constp.tile([128, nf], F32, tag="b1")
            nc.gpsimd.dma_start(b1_sb[:], b1[:])
            cw_sb = constp.tile([128, C // 128], F32, tag="cw")
            nc.gpsimd.dma_start(cw_sb[:], cw[:])

            t0 = 0
            for bi, blk in enumerate(blocks):
                ntt = blk // 128
                x_sb = xp.tile([128, nd, MAX_BLK], F32R, tag="x")
                xs0 = min(512, blk)
                nc.sync.dma_start(x_sb[:, :, :xs0], xT[bi][:, :, :xs0])
                if blk > 512:
                    nc.sync.dma_start(x_sb[:, :, 512:blk], xT[bi][:, :, 512:blk])
                h_sb = hp.tile([128, nf, MAX_BLK], F32R, tag="h")

                # ---- phase 1 ----
                for f in range(nf):
                    w1_sb = w1p.tile([128, nd, 128], F32R, tag="w1")
                    nc.sync.dma_start(w1_sb[:], w1[f])
                    s0 = 0
                    for ts in _split_subtiles(blk):
                        ph = ps1.tile([128, 512], F32, tag="ph")
                        for d in range(nd):
                            nc.tensor.matmul(
                                ph[:, :ts],
                                w1_sb[:, d, :],
                                x_sb[:, d, s0 : s0 + ts],
                                start=(d == 0),
                                stop=(d == nd - 1),
                            )
                        nc.scalar.activation(
                            h_sb[:, f, s0 : s0 + ts],
                            ph[:, :ts],
                            mybir.ActivationFunctionType.Silu,
                            bias=b1_sb[:, f : f + 1],
                        )
                        s0 += ts

                # ---- phase 2 ----
                for dh in range(2):
                    pys = [
                        ps2.tile([128, 512], F32, tag="py", name=f"py{i}")
                        for i in range(ntt)
                    ]
                    for f in range(nf):
                        w2_sb = w2p.tile([128, 512], F32R, tag="w2")
                        nc.gpsimd.dma_start(w2_sb[:], w2[f, dh])
                        for tt in range(ntt):
                            nc.tensor.matmul(
                                pys[tt][:],
                                h_sb[:, f, tt * 128 : (tt + 1) * 128],
                                w2_sb[:],
                                start=(f == 0),
                                stop=(f == nf - 1),
                            )
                    for tt in range(ntt):
                        g = t0 // 128 + tt
                        y_sb = yp.tile([128, 512], F32, tag="y")
                        nc.vector.tensor_scalar_mul(
                            y_sb[:], pys[tt][:], cw_sb[:, g : g + 1]
                        )
                        nc.sync.dma_start(
                            y[t0 + tt * 128 : t0 + (tt + 1) * 128,
                              dh * 512 : (dh + 1) * 512],
                            y_sb[:],
                        )
                t0 += blk
    nc.finalize()
    return nc


def _route(inputs, Wg, bg, k):
    """Host gate: replicate reference numerics (fp32) for routing."""
    logits = inputs.astype(np.float32) @ Wg.astype(np.float32) + bg.astype(np.float32)
    sel = np.argsort(-logits, axis=1, kind="stable")[:, :k]  # == jax.lax.top_k order
    tl = np.take_along_axis(logits, sel, axis=1).astype(np.float32)
    m = tl.max(axis=1, keepdims=True)
    e = np.exp(tl - m, dtype=np.float32)
    w = (e / e.sum(axis=1, keepdims=True)).astype(np.float32)
    return sel, w


def kernel(inputs, Wg, bg, W1, b1, W2, b2, k):
    global LAST_EXEC_TIME_NS
    k = int(np.asarray(k))
    inputs = np.ascontiguousarray(np.asarray(inputs, dtype=np.float32))
    Wg = np.asarray(Wg, dtype=np.float32)
    bg = np.asarray(bg, dtype=np.float32)
    W1 = np.asarray(W1, dtype=np.float32)
    b1 = np.asarray(b1, dtype=np.float32)
    W2 = np.asarray(W2, dtype=np.float32)
    b2 = np.asarray(b2, dtype=np.float32)

    N, D = inputs.shape
    E = Wg.shape[1]
    assert E == N_EXPERTS and D == D_MODEL and W1.shape == (E, D, D_FF)

    sel, w = _route(inputs, Wg, bg, k)

    # per-expert token lists
    idxs, wvals = [], []
    for e in range(E):
        tok, slot = np.nonzero(sel == e)
        idxs.append(tok)
        wvals.append(w[tok, slot])
    max_cnt = max(len(ix) for ix in idxs)
    C = max(((max_cnt + 127) // 128) * 128, 128)

    if MODE == "bf16_resident":
        import ml_dtypes

        wdt = ml_dtypes.bfloat16
    else:
        wdt = np.float32

    in_maps = []
    nb = len(_split_blocks(C))
    Cp = nb * MAX_BLK  # x padded to whole blocks
    for e in range(E):
        cnt = len(idxs[e])
        cwe = np.zeros((C,), dtype=np.float32)
        cwe[:cnt] = wvals[e]
        if MODE == "bf16_resident":
            xe = np.zeros((Cp, D), dtype=wdt)
            xe[:cnt] = inputs[idxs[e]].astype(wdt)
            # [Cp, D] -> [nb, 128, 8, MAX_BLK]: t=(b, t'), d=(a, p)
            xe = np.ascontiguousarray(
                xe.reshape(nb, MAX_BLK, 8, 128).transpose(0, 3, 2, 1)
            )
            w1e = np.ascontiguousarray(
                W1[e].astype(wdt).reshape(8, 128, 8, 512).transpose(2, 1, 0, 3)
            )  # [fc, p, d-chunk, f-within]
            w2e = np.ascontiguousarray(
                W2[e].astype(wdt).reshape(4, 8, 128, D).transpose(0, 2, 1, 3)
            )  # [fc, p, f-within, d]
            b1e = np.ascontiguousarray(b1[e].reshape(32, 128).T)
            cwe = np.ascontiguousarray(cwe.reshape(C // 128, 128).T)
        else:
            xe = np.zeros((Cp, D), dtype=wdt)
            xe[:cnt] = inputs[idxs[e]]
            xe = np.ascontiguousarray(
                xe.reshape(nb, MAX_BLK, 8, 128).transpose(0, 3, 2, 1)
            )
            w1e = np.ascontiguousarray(
                W1[e].reshape(8, 128, 32, 128).transpose(2, 1, 0, 3)
            )  # [f-tile, p, d-chunk, f-within]
            w2e = np.ascontiguousarray(
                W2[e].reshape(32, 128, 2, 512).transpose(0, 2, 1, 3)
            )  # [f-tile, d-half, p, d-within]
            b1e = np.ascontiguousarray(b1[e].reshape(32, 128).T)
            cwe = np.ascontiguousarray(cwe.reshape(C // 128, 128).T)
        in_maps.append(
            {"xT": xe, "w1": w1e, "b1": b1e, "w2": w2e, "cw": cwe}
        )

    key = (MODE, C)
    if key not in _NC_CACHE:
        if MODE == "bf16_resident":
            _NC_CACHE[key] = _build_nc_bf16_resident(C)
        else:
            _NC_CACHE[key] = _build_nc_f32r_stream(C)
    nc = _NC_CACHE[key]

    trace = bool(os.environ.get("BASS_TRACE"))
    res = None
    for attempt in range(3):
        try:
            res = run_bass_kernel_spmd(
                nc, in_maps, core_ids=list(range(N_CORES)), trace=trace
            )
            break
        except Exception:
            # transient NRT/device failures recover after a short pause
            if attempt == 2:
                raise
            import time

            time.sleep(20)
    LAST_EXEC_TIME_NS = getattr(res, "exec_time_ns", None)

    results = np.zeros((N, D), dtype=np.float32)
    for e in range(E):
        cnt = len(idxs[e])
        ye = np.asarray(res.results[e]["y"])[:cnt]
        # device computed w * (silu(x W1 + b1) @ W2); add the w * b2[e] term here
        results[idxs[e]] += ye + wvals[e][:, None] * b2[e][None, :]
    return results.astype(np.float32)



# revision 2
# speedup vs baseline: 1.1145x; 1.1145x over previous
"""MoE layer (N=8192, D=1024, F=4096, E=8, top-2) on 8 Trainium2 NeuronCores.

Strategy (F-split, fully load-balanced):
  - Host: gate (inputs @ Wg + bg), top-k selection, softmax combine weights,
    token gather per expert (the tiny O(N*D*E) part), final partial-sum
    combine + scatter-add + b2 term.
  - Device (SPMD): every core holds a 512-wide slice of the FFN hidden dim
    (F/8) of ALL 8 experts' weights resident in SBUF (bf16, 128 KB/part)
    and processes ALL token-expert pairs at 1/8 width:
        part_y = cw * (silu(x @ W1[e][:, cut]) + b1[e][cut]) @ W2[e][cut, :]
    mm2 contracts over F, so the 8 per-core partials simply sum on the
    host -- no cross-core communication, and the load is perfectly
    balanced regardless of routing (every core does identical work).

Per-core kernel layout (all flat [128, cols] SBUF tiles, bf16):
  w1: col = e*4096 + ft*1024 + d*128 + f    (stationary [128d x 128f] tiles)
  w2: col = e*4096 + ft*1024 + dcol         (moving [128f x 512d] tiles)
  x:  per token block (<=512 tokens, single expert): col = d*blk + t
  mm1: h[f, t]  = silu(sum_d w1 tile.T @ x tile + b1)   (psum [128f, blk])
  mm2: y[t, d]  = sum_ft h tile.T @ w2 tile             (psum [128t, 512d])
  scale rows by combine weight on VectorE (f32 psum -> bf16), DMA out.
Blocks are interleaved mm1(b+1) then mm2(b) so the PE never waits on the
silu of its own block; weights/x stream in first-use order so the first
matmul only gates on ~0.5 MB of DMA.
"""

import math
import os
import sys
import types

import numpy as np

import concourse.bass as bass
import concourse.bacc as bacc
import concourse.mybir as mybir
import concourse.tile as tile
from concourse.bass_utils import run_bass_kernel_spmd


def _ensure_ntff_hook():
    """Provide antenv.axon_hooks if the image lacks it, so trace=True (or a
    caller-set BASS_TRACE=1) degrades gracefully instead of crashing in
    run_bass_kernel_spmd."""
    try:
        import antenv.axon_hooks  # noqa: F401

        return
    except ImportError:
        pass
    hook = None
    try:
        from trn_agent_boot.trn_boot import _ntff_profile_via_ctypes

        hook = _ntff_profile_via_ctypes("/opt/axon/libaxon_pjrt.so")
    except Exception:
        hook = None
    m = types.ModuleType("antenv.axon_hooks")
    m.get_axon_ntff_profile_hook = lambda: hook
    m.set_axon_ntff_profile_hook = lambda h: None
    sys.modules["antenv.axon_hooks"] = m
    try:
        import antenv

        antenv.axon_hooks = m
    except ImportError:
        pass


_ensure_ntff_hook()

F32 = mybir.dt.float32
BF16 = mybir.dt.bfloat16

N_TOK = 8192
D_MODEL = 1024
D_FF = 4096
N_EXPERTS = 8
N_CORES = 8
FCUT = D_FF // N_CORES  # 512: f-columns resident per core
NFT = FCUT // 128       # 4 f-tiles per expert per core
WCOLS = N_EXPERTS * NFT * 1024  # 32768 weight cols per tensor

LAST_EXEC_TIME_NS = None
_NC_CACHE = {}


def _blocks_from_tiles(tiles):
    """Token blocks (expert, blk): each block is one expert's tokens,
    <=512 of them. The very first block is 128 tokens so the first matmul
    gates on a minimal DMA."""
    blocks = []
    first = True
    for e, nt in enumerate(tiles):
        g = nt * 128
        if g == 0:
            continue
        if first:
            blocks.append((e, 128))
            g -= 128
            first = False
        while g:
            b = min(512, g)
            blocks.append((e, b))
            g -= b
    return blocks


def _build_nc(tiles):
    tiles = list(tiles)
    blocks = _blocks_from_tiles(tiles)
    nb = len(blocks)
    T = sum(tiles) * 128

    nc = bacc.Bacc("TRN2", target_bir_lowering=False, debug=False)
    w1 = nc.declare_dram_parameter("w1", [128, WCOLS], BF16, isOutput=False)
    w2 = nc.declare_dram_parameter("w2", [128, WCOLS], BF16, isOutput=False)
    b1 = nc.declare_dram_parameter("b1", [128, N_EXPERTS * NFT], F32, isOutput=False)
    cw = nc.declare_dram_parameter("cw", [128, T // 128], F32, isOutput=False)
    x = nc.declare_dram_parameter("x", [128, 8 * T], BF16, isOutput=False)
    y = nc.declare_dram_parameter("y", [T, D_MODEL], BF16, isOutput=True)

    # per-block x column offset and global tile index
    xoff = []
    tile0 = []
    o = t = 0
    for e, blk in blocks:
        xoff.append(o)
        tile0.append(t)
        o += 8 * blk
        t += blk // 128

    PF = 3  # x prefetch distance in blocks

    with tile.TileContext(nc) as tc:
        with (
            tc.tile_pool(name="wres", bufs=1) as wres,
            tc.tile_pool(name="const", bufs=1) as constp,
            tc.tile_pool(name="xp", bufs=PF + 1) as xp,
            tc.tile_pool(name="hp", bufs=3) as hp,
            tc.tile_pool(name="yp", bufs=4) as yp,
            tc.tile_pool(name="ps1", bufs=3, space="PSUM") as ps1,
            tc.tile_pool(name="ps2", bufs=4, space="PSUM") as ps2,
        ):
            w1_sb = wres.tile([128, WCOLS], BF16, tag="w1")
            w2_sb = wres.tile([128, WCOLS], BF16, tag="w2")
            b1_sb = constp.tile([128, N_EXPERTS * NFT], F32, tag="b1")
            cw_sb = constp.tile([128, T // 128], F32, tag="cw")

            x_tiles = [None] * nb
            h_tiles = [None] * nb

            def dma_x(bi):
                e, blk = blocks[bi]
                xt = xp.tile([128, 8 * 512], BF16, tag="x")
                nc.sync.dma_start(xt[:, : 8 * blk], x[:, xoff[bi] : xoff[bi] + 8 * blk])
                x_tiles[bi] = xt

            def dma_w1(e, split):
                if split:
                    for ft in range(NFT):
                        a = e * 4096 + ft * 1024
                        nc.sync.dma_start(w1_sb[:, a : a + 1024], w1[:, a : a + 1024])
                else:
                    a = e * 4096
                    nc.sync.dma_start(w1_sb[:, a : a + 4096], w1[:, a : a + 4096])

            def dma_w2(e):
                a = e * 4096
                nc.sync.dma_start(w2_sb[:, a : a + 4096], w2[:, a : a + 4096])

            # --- initial DMA schedule: gate the first matmul on w1[e0,ft0]+x[b0]
            e0 = blocks[0][0]
            loaded = {e0}
            dma_w1(e0, split=True)  # ft0 lands first (256 KB)
            dma_x(0)
            nc.sync.dma_start(b1_sb[:], b1[:])
            dma_w2(e0)
            nc.sync.dma_start(cw_sb[:], cw[:])
            for bi in range(1, min(PF, nb)):
                ee = blocks[bi][0]
                if ee not in loaded:
                    loaded.add(ee)
                    dma_w1(ee, split=False)
                    dma_w2(ee)
                dma_x(bi)

            def do_mm1(bi):
                e, blk = blocks[bi]
                x_sb = x_tiles[bi]
                h_sb = hp.tile([128, NFT * 512], BF16, tag="h")
                h_tiles[bi] = h_sb
                for ft in range(NFT):
                    ph = ps1.tile([128, 512], F32, tag="ph")
                    wa = e * 4096 + ft * 1024
                    for d in range(8):
                        nc.tensor.matmul(
                            ph[:, :blk],
                            w1_sb[:, wa + d * 128 : wa + (d + 1) * 128],
                            x_sb[:, d * blk : (d + 1) * blk],
                            start=(d == 0),
                            stop=(d == 7),
                        )
                    nc.scalar.activation(
                        h_sb[:, ft * blk : ft * blk + blk],
                        ph[:, :blk],
                        mybir.ActivationFunctionType.Silu,
                        bias=b1_sb[:, e * NFT + ft : e * NFT + ft + 1],
                    )

            def do_mm2(bi):
                e, blk = blocks[bi]
                h_sb = h_tiles[bi]
                t0 = tile0[bi]
                for tt in range(blk // 128):
                    y_sb = yp.tile([128, D_MODEL], BF16, tag="y")
                    for dh in range(2):
                        py = ps2.tile([128, 512], F32, tag="py")
                        for ft in range(NFT):
                            wa = e * 4096 + ft * 1024
                            nc.tensor.matmul(
                                py[:],
                                h_sb[:, ft * blk + tt * 128 : ft * blk + (tt + 1) * 128],
                                w2_sb[:, wa + dh * 512 : wa + (dh + 1) * 512],
                                start=(ft == 0),
                                stop=(ft == NFT - 1),
                            )
                        nc.vector.tensor_scalar_mul(
                            y_sb[:, dh * 512 : (dh + 1) * 512],
                            py[:],
                            cw_sb[:, t0 + tt : t0 + tt + 1],
                        )
                    nc.sync.dma_start(
                        y[(t0 + tt) * 128 : (t0 + tt + 1) * 128, :], y_sb[:]
                    )

            for bi in range(nb):
                # prefetch x (and weights on expert change) PF blocks ahead
                pf = bi + PF
                if pf < nb:
                    ee = blocks[pf][0]
                    if ee not in loaded:
                        loaded.add(ee)
                        dma_w1(ee, split=False)
                        dma_w2(ee)
                    dma_x(pf)
                do_mm1(bi)
                if bi > 0:
                    do_mm2(bi - 1)
            do_mm2(nb - 1)
    nc.finalize()
    return nc


def _route(inputs, Wg, bg, k):
    """Host gate: replicate reference numerics (fp32) for routing."""
    logits = inputs.astype(np.float32) @ Wg.astype(np.float32) + bg.astype(np.float32)
    sel = np.argsort(-logits, axis=1, kind="stable")[:, :k]  # == jax.lax.top_k order
    tl = np.take_along_axis(logits, sel, axis=1).astype(np.float32)
    m = tl.max(axis=1, keepdims=True)
    e = np.exp(tl - m, dtype=np.float32)
    w = (e / e.sum(axis=1, keepdims=True)).astype(np.float32)
    return sel, w


def _prepare(inputs, W1, b1, W2, idxs, wvals, tiles, blocks):
    """Build the device input arrays (shared x/cw + per-core weight cuts)."""
    import ml_dtypes

    bf16 = ml_dtypes.bfloat16
    E = N_EXPERTS
    T = sum(tiles) * 128

    xg = np.zeros((T, D_MODEL), dtype=bf16)
    cwf = np.zeros((T,), dtype=np.float32)
    expert_off = []
    off = 0
    for e in range(E):
        cnt = len(idxs[e])
        expert_off.append(off)
        xg[off : off + cnt] = inputs[idxs[e]].astype(bf16)
        cwf[off : off + cnt] = wvals[e]
        off += tiles[e] * 128

    xcols = np.empty((128, 8 * T), dtype=bf16)
    t0 = c0 = 0
    for e, blk in blocks:
        xb = xg[t0 : t0 + blk].reshape(blk, 8, 128).transpose(2, 1, 0)
        xcols[:, c0 : c0 + 8 * blk] = xb.reshape(128, 8 * blk)
        t0 += blk
        c0 += 8 * blk
    cwh = np.ascontiguousarray(cwf.reshape(T // 128, 128).T)

    in_maps = []
    for core in range(N_CORES):
        c0f = core * FCUT
        # (e, d, p, ft, f) -> (p, e, ft, d, f)
        w1h = np.ascontiguousarray(
            W1[:, :, c0f : c0f + FCUT]
            .astype(bf16)
            .reshape(E, 8, 128, NFT, 128)
            .transpose(2, 0, 3, 1, 4)
            .reshape(128, WCOLS)
        )
        # (e, ft, p, d) -> (p, e, ft, d)
        w2h = np.ascontiguousarray(
            W2[:, c0f : c0f + FCUT, :]
            .astype(bf16)
            .reshape(E, NFT, 128, D_MODEL)
            .transpose(2, 0, 1, 3)
            .reshape(128, WCOLS)
        )
        # (e, ft, p) -> (p, e, ft)
        b1h = np.ascontiguousarray(
            b1[:, c0f : c0f + FCUT]
            .reshape(E, NFT, 128)
            .transpose(2, 0, 1)
            .reshape(128, E * NFT)
        ).astype(np.float32)
        in_maps.append({"w1": w1h, "w2": w2h, "b1": b1h, "cw": cwh, "x": xcols})
    return in_maps, expert_off


def kernel(inputs, Wg, bg, W1, b1, W2, b2, k):
    global LAST_EXEC_TIME_NS
    k = int(np.asarray(k))
    inputs = np.ascontiguousarray(np.asarray(inputs, dtype=np.float32))
    Wg = np.asarray(Wg, dtype=np.float32)
    bg = np.asarray(bg, dtype=np.float32)
    W1 = np.asarray(W1, dtype=np.float32)
    b1 = np.asarray(b1, dtype=np.float32)
    W2 = np.asarray(W2, dtype=np.float32)
    b2 = np.asarray(b2, dtype=np.float32)

    N, D = inputs.shape
    E = Wg.shape[1]
    assert E == N_EXPERTS and D == D_MODEL and W1.shape == (E, D, D_FF)

    sel, w = _route(inputs, Wg, bg, k)

    idxs, wvals = [], []
    for e in range(E):
        tok, slot = np.nonzero(sel == e)
        idxs.append(tok)
        wvals.append(w[tok, slot])
    tiles = [(len(ix) + 127) // 128 for ix in idxs]
    blocks = _blocks_from_tiles(tiles)

    in_maps, expert_off = _prepare(inputs, W1, b1, W2, idxs, wvals, tiles, blocks)

    key = tuple(tiles)
    if key not in _NC_CACHE:
        _NC_CACHE[key] = _build_nc(tiles)
    nc = _NC_CACHE[key]

    trace = bool(os.environ.get("BASS_TRACE"))
    res = None
    for attempt in range(3):
        try:
            res = run_bass_kernel_spmd(
                nc, in_maps, core_ids=list(range(N_CORES)), trace=trace
            )
            break
        except Exception:
            # transient NRT/device failures recover after a short pause
            if attempt == 2:
                raise
            import time

            time.sleep(20)
    LAST_EXEC_TIME_NS = getattr(res, "exec_time_ns", None)

    ysum = np.zeros((sum(tiles) * 128, D_MODEL), dtype=np.float32)
    for c in range(N_CORES):
        ysum += np.asarray(res.results[c]["y"]).astype(np.float32)

    results = np.zeros((N, D), dtype=np.float32)
    for e in range(E):
        cnt = len(idxs[e])
        o = expert_off[e]
        # device computed cw * (silu(x W1 + b1) @ W2); add cw * b2[e] here
        results[idxs[e]] += ysum[o : o + cnt] + wvals[e][:, None] * b2[e][None, :]
    return results.astype(np.float32)


# revision 11
# speedup vs baseline: 1.1303x; 1.0141x over previous
"""MoE layer (N=8192, D=1024, F=4096, E=8, top-2) on 8 Trainium2 NeuronCores.

Strategy (F-split, fully load-balanced):
  - Host: gate (inputs @ Wg + bg), top-k selection, softmax combine weights,
    token gather per expert (the tiny O(N*D*E) part), final partial-sum
    combine + scatter-add + b2 term.
  - Device (SPMD): every core holds a 512-wide slice of the FFN hidden dim
    (F/8) of ALL 8 experts' weights resident in SBUF (bf16, 128 KB/part)
    and processes ALL token-expert pairs at 1/8 width:
        part_y = cw * (silu(x @ W1[e][:, cut]) + b1[e][cut]) @ W2[e][cut, :]
    mm2 contracts over F, so the 8 per-core partials simply sum on the
    host -- no cross-core communication, and the load is perfectly
    balanced regardless of routing (every core does identical work).

Per-core kernel layout (all flat [128, cols] SBUF tiles, bf16):
  w1: col = e*4096 + ft*1024 + d*128 + f    (stationary [128d x 128f] tiles)
  w2: col = e*4096 + ft*1024 + dcol         (moving [128f x 512d] tiles)
  x:  per token block (<=512 tokens, single expert): col = d*blk + t
  mm1: h[f, t]  = silu(sum_d w1 tile.T @ x tile + b1)   (psum [128f, blk])
  mm2: y[t, d]  = sum_ft h tile.T @ w2 tile             (psum [128t, 512d])
  scale rows by combine weight on VectorE (f32 psum -> bf16), DMA out.
Blocks are interleaved mm1(b+1) then mm2(b) so the PE never waits on the
silu of its own block; weights/x stream in first-use order so the first
matmul only gates on ~0.5 MB of DMA.
"""

import math
import os
import sys
import types

import numpy as np

import concourse.bass as bass
import concourse.bacc as bacc
import concourse.mybir as mybir
import concourse.tile as tile
from concourse.bass_utils import run_bass_kernel_spmd


def _ensure_ntff_hook():
    """Provide antenv.axon_hooks if the image lacks it, so trace=True (or a
    caller-set BASS_TRACE=1) degrades gracefully instead of crashing in
    run_bass_kernel_spmd."""
    try:
        import antenv.axon_hooks  # noqa: F401

        return
    except ImportError:
        pass
    hook = None
    try:
        from trn_agent_boot.trn_boot import _ntff_profile_via_ctypes

        hook = _ntff_profile_via_ctypes("/opt/axon/libaxon_pjrt.so")
    except Exception:
        hook = None
    m = types.ModuleType("antenv.axon_hooks")
    m.get_axon_ntff_profile_hook = lambda: hook
    m.set_axon_ntff_profile_hook = lambda h: None
    sys.modules["antenv.axon_hooks"] = m
    try:
        import antenv

        antenv.axon_hooks = m
    except ImportError:
        pass


_ensure_ntff_hook()

F32 = mybir.dt.float32
BF16 = mybir.dt.bfloat16

N_TOK = 8192
D_MODEL = 1024
D_FF = 4096
N_EXPERTS = 8
N_CORES = 8
FCUT = D_FF // N_CORES  # 512: f-columns resident per core
NFT = FCUT // 128       # 4 f-tiles per expert per core
WCOLS = N_EXPERTS * NFT * 1024  # 32768 weight cols per tensor

LAST_EXEC_TIME_NS = None
_NC_CACHE = {}


def _blocks_from_tiles(counts):
    """Token blocks (expert, blk): each block is one expert's tokens
    (exact count, no padding -- only mm2's 128-token tiles are ragged),
    <=512 of them. The very first block is 128 tokens so the first matmul
    gates on a minimal DMA."""
    blocks = []
    first = True
    for e, g in enumerate(counts):
        if g == 0:
            continue
        if first and g > 128:
            blocks.append((e, 128))
            g -= 128
            first = False
        while g:
            b = min(512, g)
            blocks.append((e, b))
            g -= b
    return blocks


def _build_nc(counts):
    counts = list(counts)
    blocks = _blocks_from_tiles(counts)
    nb = len(blocks)
    T = sum(counts)
    ncw = sum((blk + 127) // 128 for _, blk in blocks)

    nc = bacc.Bacc("TRN2", target_bir_lowering=False, debug=False)
    w1 = nc.declare_dram_parameter("w1", [128, WCOLS], BF16, isOutput=False)
    w2 = nc.declare_dram_parameter("w2", [128, WCOLS], BF16, isOutput=False)
    b1 = nc.declare_dram_parameter("b1", [128, N_EXPERTS * NFT], F32, isOutput=False)
    cw = nc.declare_dram_parameter("cw", [128, ncw], F32, isOutput=False)
    x = nc.declare_dram_parameter("x", [128, 8 * T], BF16, isOutput=False)
    y = nc.declare_dram_parameter("y", [T, D_MODEL], BF16, isOutput=True)

    # per-block x column offset, token offset, and combine-weight column
    xoff = []
    toff = []
    tile0 = []
    o = t = tc_ = 0
    for e, blk in blocks:
        xoff.append(o)
        toff.append(t)
        tile0.append(tc_)
        o += 8 * blk
        t += blk
        tc_ += (blk + 127) // 128

    PF = 3  # x prefetch distance in blocks

    with tile.TileContext(nc) as tc:
        with (
            tc.tile_pool(name="wres", bufs=1) as wres,
            tc.tile_pool(name="const", bufs=1) as constp,
            tc.tile_pool(name="xp", bufs=PF + 1) as xp,
            tc.tile_pool(name="hp", bufs=3) as hp,
            tc.tile_pool(name="yp", bufs=4) as yp,
            tc.tile_pool(name="ps1", bufs=3, space="PSUM") as ps1,
            tc.tile_pool(name="ps2", bufs=4, space="PSUM") as ps2,
        ):
            w1_sb = wres.tile([128, WCOLS], BF16, tag="w1")
            w2_sb = wres.tile([128, WCOLS], BF16, tag="w2")
            b1_sb = constp.tile([128, N_EXPERTS * NFT], F32, tag="b1")
            cw_sb = constp.tile([128, ncw], F32, tag="cw")

            x_tiles = [None] * nb
            h_tiles = [None] * nb

            def dma_x(bi, eng=None):
                e, blk = blocks[bi]
                xt = xp.tile([128, 8 * 512], BF16, tag="x")
                (eng or nc.sync).dma_start(
                    xt[:, : 8 * blk], x[:, xoff[bi] : xoff[bi] + 8 * blk]
                )
                x_tiles[bi] = xt

            def dma_w1(e, split):
                if split:
                    for ft in range(NFT):
                        a = e * 4096 + ft * 1024
                        nc.sync.dma_start(w1_sb[:, a : a + 1024], w1[:, a : a + 1024])
                else:
                    a = e * 4096
                    nc.sync.dma_start(w1_sb[:, a : a + 4096], w1[:, a : a + 4096])

            def dma_w2(e):
                a = e * 4096
                nc.sync.dma_start(w2_sb[:, a : a + 4096], w2[:, a : a + 4096])

            # --- initial DMA schedule: gate the first matmul on w1[e0,ft0]+x[b0]
            e0 = blocks[0][0]
            loaded = {e0}
            # x[b0] on the scalar queue so it transfers in parallel with
            # w1[e0,ft0] on the sync queue -- the first matmul gates on both
            dma_x(0, eng=nc.scalar)
            dma_w1(e0, split=True)  # ft0 lands first (256 KB)
            nc.sync.dma_start(b1_sb[:], b1[:])
            dma_w2(e0)
            nc.sync.dma_start(cw_sb[:], cw[:])
            for bi in range(1, min(PF, nb)):
                ee = blocks[bi][0]
                if ee not in loaded:
                    loaded.add(ee)
                    dma_w1(ee, split=False)
                    dma_w2(ee)
                dma_x(bi)

            def do_mm1(bi):
                e, blk = blocks[bi]
                x_sb = x_tiles[bi]
                h_sb = hp.tile([128, NFT * 512], BF16, tag="h")
                h_tiles[bi] = h_sb
                for ft in range(NFT):
                    ph = ps1.tile([128, 512], F32, tag="ph")
                    wa = e * 4096 + ft * 1024
                    for d in range(8):
                        nc.tensor.matmul(
                            ph[:, :blk],
                            w1_sb[:, wa + d * 128 : wa + (d + 1) * 128],
                            x_sb[:, d * blk : (d + 1) * blk],
                            start=(d == 0),
                            stop=(d == 7),
                        )
                    nc.scalar.activation(
                        h_sb[:, ft * blk : ft * blk + blk],
                        ph[:, :blk],
                        mybir.ActivationFunctionType.Silu,
                        bias=b1_sb[:, e * NFT + ft : e * NFT + ft + 1],
                    )

            def do_mm2(bi):
                e, blk = blocks[bi]
                h_sb = h_tiles[bi]
                t0 = toff[bi]
                c0 = tile0[bi]
                for tt in range((blk + 127) // 128):
                    rows = min(128, blk - tt * 128)
                    y_sb = yp.tile([128, D_MODEL], BF16, tag="y")
                    for dh in range(2):
                        py = ps2.tile([128, 512], F32, tag="py")
                        for ft in range(NFT):
                            wa = e * 4096 + ft * 1024
                            nc.tensor.matmul(
                                py[:rows, :],
                                h_sb[:, ft * blk + tt * 128 : ft * blk + tt * 128 + rows],
                                w2_sb[:, wa + dh * 512 : wa + (dh + 1) * 512],
                                start=(ft == 0),
                                stop=(ft == NFT - 1),
                            )
                        nc.vector.tensor_scalar_mul(
                            y_sb[:rows, dh * 512 : (dh + 1) * 512],
                            py[:rows, :],
                            cw_sb[:rows, c0 + tt : c0 + tt + 1],
                        )
                    nc.sync.dma_start(
                        y[t0 + tt * 128 : t0 + tt * 128 + rows, :], y_sb[:rows, :]
                    )

            for bi in range(nb):
                # prefetch x (and weights on expert change) PF blocks ahead
                pf = bi + PF
                if pf < nb:
                    ee = blocks[pf][0]
                    if ee not in loaded:
                        loaded.add(ee)
                        dma_w1(ee, split=False)
                        dma_w2(ee)
                    dma_x(pf)
                do_mm1(bi)
                if bi > 0:
                    do_mm2(bi - 1)
            do_mm2(nb - 1)
    nc.finalize()
    return nc


def _route(inputs, Wg, bg, k):
    """Host gate: replicate reference numerics (fp32) for routing."""
    logits = inputs.astype(np.float32) @ Wg.astype(np.float32) + bg.astype(np.float32)
    sel = np.argsort(-logits, axis=1, kind="stable")[:, :k]  # == jax.lax.top_k order
    tl = np.take_along_axis(logits, sel, axis=1).astype(np.float32)
    m = tl.max(axis=1, keepdims=True)
    e = np.exp(tl - m, dtype=np.float32)
    w = (e / e.sum(axis=1, keepdims=True)).astype(np.float32)
    return sel, w


def _prepare(inputs, W1, b1, W2, idxs, wvals, counts, blocks):
    """Build the device input arrays (shared x/cw + per-core weight cuts)."""
    import ml_dtypes

    bf16 = ml_dtypes.bfloat16
    E = N_EXPERTS
    T = sum(counts)

    xg = np.empty((T, D_MODEL), dtype=bf16)
    cwf = np.empty((T,), dtype=np.float32)
    expert_off = []
    off = 0
    for e in range(E):
        cnt = counts[e]
        expert_off.append(off)
        xg[off : off + cnt] = inputs[idxs[e]].astype(bf16)
        cwf[off : off + cnt] = wvals[e]
        off += cnt

    xcols = np.empty((128, 8 * T), dtype=bf16)
    ncw = sum((blk + 127) // 128 for _, blk in blocks)
    cwh = np.zeros((128, ncw), dtype=np.float32)
    t0 = c0 = cc = 0
    for e, blk in blocks:
        xb = xg[t0 : t0 + blk].reshape(blk, 8, 128).transpose(2, 1, 0)
        xcols[:, c0 : c0 + 8 * blk] = xb.reshape(128, 8 * blk)
        for tt in range((blk + 127) // 128):
            rows = min(128, blk - tt * 128)
            cwh[:rows, cc] = cwf[t0 + tt * 128 : t0 + tt * 128 + rows]
            cc += 1
        t0 += blk
        c0 += 8 * blk

    in_maps = []
    for core in range(N_CORES):
        c0f = core * FCUT
        # (e, d, p, ft, f) -> (p, e, ft, d, f)
        w1h = np.ascontiguousarray(
            W1[:, :, c0f : c0f + FCUT]
            .astype(bf16)
            .reshape(E, 8, 128, NFT, 128)
            .transpose(2, 0, 3, 1, 4)
            .reshape(128, WCOLS)
        )
        # (e, ft, p, d) -> (p, e, ft, d)
        w2h = np.ascontiguousarray(
            W2[:, c0f : c0f + FCUT, :]
            .astype(bf16)
            .reshape(E, NFT, 128, D_MODEL)
            .transpose(2, 0, 1, 3)
            .reshape(128, WCOLS)
        )
        # (e, ft, p) -> (p, e, ft)
        b1h = np.ascontiguousarray(
            b1[:, c0f : c0f + FCUT]
            .reshape(E, NFT, 128)
            .transpose(2, 0, 1)
            .reshape(128, E * NFT)
        ).astype(np.float32)
        in_maps.append({"w1": w1h, "w2": w2h, "b1": b1h, "cw": cwh, "x": xcols})
    return in_maps, expert_off


def kernel(inputs, Wg, bg, W1, b1, W2, b2, k):
    global LAST_EXEC_TIME_NS
    k = int(np.asarray(k))
    inputs = np.ascontiguousarray(np.asarray(inputs, dtype=np.float32))
    Wg = np.asarray(Wg, dtype=np.float32)
    bg = np.asarray(bg, dtype=np.float32)
    W1 = np.asarray(W1, dtype=np.float32)
    b1 = np.asarray(b1, dtype=np.float32)
    W2 = np.asarray(W2, dtype=np.float32)
    b2 = np.asarray(b2, dtype=np.float32)

    N, D = inputs.shape
    E = Wg.shape[1]
    assert E == N_EXPERTS and D == D_MODEL and W1.shape == (E, D, D_FF)

    sel, w = _route(inputs, Wg, bg, k)

    idxs, wvals = [], []
    for e in range(E):
        tok, slot = np.nonzero(sel == e)
        idxs.append(tok)
        wvals.append(w[tok, slot])
    counts = [len(ix) for ix in idxs]
    blocks = _blocks_from_tiles(counts)

    in_maps, expert_off = _prepare(inputs, W1, b1, W2, idxs, wvals, counts, blocks)

    key = tuple(counts)
    if key not in _NC_CACHE:
        _NC_CACHE[key] = _build_nc(counts)
    nc = _NC_CACHE[key]

    trace = bool(os.environ.get("BASS_TRACE"))
    res = None
    for attempt in range(3):
        try:
            res = run_bass_kernel_spmd(
                nc, in_maps, core_ids=list(range(N_CORES)), trace=trace
            )
            break
        except Exception:
            # transient NRT/device failures recover after a short pause
            if attempt == 2:
                raise
            import time

            time.sleep(20)
    LAST_EXEC_TIME_NS = getattr(res, "exec_time_ns", None)

    ysum = np.zeros((sum(counts), D_MODEL), dtype=np.float32)
    for c in range(N_CORES):
        ysum += np.asarray(res.results[c]["y"]).astype(np.float32)

    results = np.zeros((N, D), dtype=np.float32)
    for e in range(E):
        cnt = counts[e]
        o = expert_off[e]
        # device computed cw * (silu(x W1 + b1) @ W2); add cw * b2[e] here
        results[idxs[e]] += ysum[o : o + cnt] + wvals[e][:, None] * b2[e][None, :]
    return results.astype(np.float32)


# revision 20
# speedup vs baseline: 1.1511x; 1.0184x over previous
"""MoE layer (N=8192, D=1024, F=4096, E=8, top-2) on 8 Trainium2 NeuronCores.

Strategy (F-split, fully load-balanced):
  - Host: gate (inputs @ Wg + bg), top-k selection, softmax combine weights,
    token gather per expert (the tiny O(N*D*E) part), final partial-sum
    combine + scatter-add + b2 term.
  - Device (SPMD): every core holds a 512-wide slice of the FFN hidden dim
    (F/8) of ALL 8 experts' weights resident in SBUF (bf16, 128 KB/part)
    and processes ALL token-expert pairs at 1/8 width:
        part_y = cw * (silu(x @ W1[e][:, cut]) + b1[e][cut]) @ W2[e][cut, :]
    mm2 contracts over F, so the 8 per-core partials simply sum on the
    host -- no cross-core communication, and the load is perfectly
    balanced regardless of routing (every core does identical work).

Per-core kernel layout (all flat [128, cols] SBUF tiles, bf16):
  w1: col = e*4096 + ft*1024 + d*128 + f    (stationary [128d x 128f] tiles)
  w2: col = e*4096 + ft*1024 + dcol         (moving [128f x 512d] tiles)
  x:  per token block (<=512 tokens, single expert): col = d*blk + t
  mm1: h[f, t]  = silu(sum_d w1 tile.T @ x tile + b1)   (psum [128f, blk])
  mm2: y[t, d]  = sum_ft h tile.T @ w2 tile             (psum [128t, 512d])
  scale rows by combine weight on VectorE (f32 psum -> bf16), DMA out.
Blocks are interleaved mm1(b+1) then mm2(b) so the PE never waits on the
silu of its own block; weights/x stream in first-use order so the first
matmul only gates on ~0.5 MB of DMA.
"""

import math
import os
import sys
import types

import numpy as np

import concourse.bass as bass
import concourse.bacc as bacc
import concourse.mybir as mybir
import concourse.tile as tile
from concourse.bass_utils import run_bass_kernel_spmd


def _ensure_ntff_hook():
    """Provide antenv.axon_hooks if the image lacks it, so trace=True (or a
    caller-set BASS_TRACE=1) degrades gracefully instead of crashing in
    run_bass_kernel_spmd."""
    try:
        import antenv.axon_hooks  # noqa: F401

        return
    except ImportError:
        pass
    hook = None
    try:
        from trn_agent_boot.trn_boot import _ntff_profile_via_ctypes

        hook = _ntff_profile_via_ctypes("/opt/axon/libaxon_pjrt.so")
    except Exception:
        hook = None
    m = types.ModuleType("antenv.axon_hooks")
    m.get_axon_ntff_profile_hook = lambda: hook
    m.set_axon_ntff_profile_hook = lambda h: None
    sys.modules["antenv.axon_hooks"] = m
    try:
        import antenv

        antenv.axon_hooks = m
    except ImportError:
        pass


_ensure_ntff_hook()

F32 = mybir.dt.float32
BF16 = mybir.dt.bfloat16

N_TOK = 8192
D_MODEL = 1024
D_FF = 4096
N_EXPERTS = 8
N_CORES = 8
FCUT = D_FF // N_CORES  # 512: f-columns resident per core
NFT = FCUT // 128       # 4 f-tiles per expert per core
WCOLS = N_EXPERTS * NFT * 1024  # 32768 weight cols per tensor

LAST_EXEC_TIME_NS = None
_NC_CACHE = {}


def _blocks_from_tiles(counts):
    """Token blocks (expert, blk): each block is one expert's tokens
    (exact count, no padding anywhere), <=512 of them, split as evenly as
    possible. The very first block is 256 tokens so the first matmul
    gates on a minimal DMA."""
    blocks = []
    first = True
    for e, g in enumerate(counts):
        if g == 0:
            continue
        if first and g > 256:
            blocks.append((e, 256))
            g -= 256
            first = False
        nsub = (g + 511) // 512
        lo = g // nsub
        hi_cnt = g - lo * nsub  # hi_cnt blocks of (lo+1), rest of lo
        for i in range(nsub):
            blocks.append((e, lo + 1 if i < hi_cnt else lo))
    return blocks


def _build_nc(counts):
    counts = list(counts)
    blocks = _blocks_from_tiles(counts)
    nb = len(blocks)
    T = sum(counts)

    nc = bacc.Bacc("TRN2", target_bir_lowering=False, debug=False)
    w1 = nc.declare_dram_parameter("w1", [128, WCOLS], BF16, isOutput=False)
    w2 = nc.declare_dram_parameter("w2", [128, WCOLS], BF16, isOutput=False)
    b1 = nc.declare_dram_parameter("b1", [128, N_EXPERTS * NFT], F32, isOutput=False)
    x = nc.declare_dram_parameter("x", [128, 8 * T], BF16, isOutput=False)
    # y is stored d-major: 8 planes of [128 d, T tokens] (unscaled partials;
    # the combine weight is applied on the host during the partial sum)
    y = nc.declare_dram_parameter("y", [8, 128, T], BF16, isOutput=True)

    # per-block x column offset and token offset
    xoff = []
    toff = []
    o = t = 0
    for e, blk in blocks:
        xoff.append(o)
        toff.append(t)
        o += 8 * blk
        t += blk

    PF = 3  # x prefetch distance in blocks

    with tile.TileContext(nc) as tc:
        with (
            tc.tile_pool(name="wres", bufs=1) as wres,
            tc.tile_pool(name="const", bufs=1) as constp,
            tc.tile_pool(name="xp", bufs=PF + 1) as xp,
            tc.tile_pool(name="hp", bufs=3) as hp,
            tc.tile_pool(name="yp", bufs=6) as yp,
            tc.tile_pool(name="ps1", bufs=3, space="PSUM") as ps1,
            tc.tile_pool(name="ps2", bufs=4, space="PSUM") as ps2,
        ):
            w1_sb = wres.tile([128, WCOLS], BF16, tag="w1")
            w2_sb = wres.tile([128, WCOLS], BF16, tag="w2")
            b1_sb = constp.tile([128, N_EXPERTS * NFT], F32, tag="b1")

            x_tiles = [None] * nb
            h_tiles = [None] * nb

            def dma_x(bi, eng=None):
                e, blk = blocks[bi]
                xt = xp.tile([128, 8 * 512], BF16, tag="x")
                (eng or nc.sync).dma_start(
                    xt[:, : 8 * blk], x[:, xoff[bi] : xoff[bi] + 8 * blk]
                )
                x_tiles[bi] = xt

            def dma_w1(e, split):
                if split:
                    for ft in range(NFT):
                        a = e * 4096 + ft * 1024
                        nc.sync.dma_start(w1_sb[:, a : a + 1024], w1[:, a : a + 1024])
                else:
                    a = e * 4096
                    nc.sync.dma_start(w1_sb[:, a : a + 4096], w1[:, a : a + 4096])

            def dma_w2(e):
                a = e * 4096
                nc.sync.dma_start(w2_sb[:, a : a + 4096], w2[:, a : a + 4096])

            # --- initial DMA schedule: gate the first matmul on w1[e0,ft0]+x[b0]
            e0 = blocks[0][0]
            loaded = {e0}
            # x[b0] on the scalar queue so it transfers in parallel with
            # w1[e0] on the sync queue -- the first matmul gates on both.
            # x[b1] is needed ~2us after the gate, so it precedes w2[e0].
            dma_x(0, eng=nc.scalar)
            dma_w1(e0, split=True)
            nc.sync.dma_start(b1_sb[:], b1[:])
            if nb > 1:
                dma_x(1)
            dma_w2(e0)
            for bi in range(2, min(PF, nb)):
                ee = blocks[bi][0]
                if ee not in loaded:
                    loaded.add(ee)
                    dma_w1(ee, split=False)
                    dma_w2(ee)
                dma_x(bi)

            def do_mm1(bi):
                e, blk = blocks[bi]
                x_sb = x_tiles[bi]
                h_sb = hp.tile([128, NFT * 512], BF16, tag="h")
                h_tiles[bi] = h_sb
                for ft in range(NFT):
                    ph = ps1.tile([128, 512], F32, tag="ph")
                    wa = e * 4096 + ft * 1024
                    for d in range(8):
                        nc.tensor.matmul(
                            ph[:, :blk],
                            w1_sb[:, wa + d * 128 : wa + (d + 1) * 128],
                            x_sb[:, d * blk : (d + 1) * blk],
                            start=(d == 0),
                            stop=(d == 7),
                        )
                    nc.scalar.activation(
                        h_sb[:, ft * blk : ft * blk + blk],
                        ph[:, :blk],
                        mybir.ActivationFunctionType.Silu,
                        bias=b1_sb[:, e * NFT + ft : e * NFT + ft + 1],
                    )

            def do_mm2(bi):
                # swapped roles: stationary = w2 [128f, 128d] tiles, moving =
                # h [128f, blk] -> psum [128d, blk]: cost scales with the
                # exact token count, no 128-token tile quantization.
                e, blk = blocks[bi]
                h_sb = h_tiles[bi]
                t0 = toff[bi]
                for dt in range(8):
                    py = ps2.tile([128, 512], F32, tag="py")
                    for ft in range(NFT):
                        wa = e * 4096 + ft * 1024
                        nc.tensor.matmul(
                            py[:, :blk],
                            w2_sb[:, wa + dt * 128 : wa + (dt + 1) * 128],
                            h_sb[:, ft * blk : ft * blk + blk],
                            start=(ft == 0),
                            stop=(ft == NFT - 1),
                        )
                    y_sb = yp.tile([128, 512], BF16, tag="y")
                    nc.vector.tensor_scalar_mul(y_sb[:, :blk], py[:, :blk], 1.0)
                    nc.sync.dma_start(y[dt][:, t0 : t0 + blk], y_sb[:, :blk])

            for bi in range(nb):
                # prefetch x (and weights on expert change) PF blocks ahead
                pf = bi + PF
                if pf < nb:
                    ee = blocks[pf][0]
                    if ee not in loaded:
                        loaded.add(ee)
                        dma_w1(ee, split=False)
                        dma_w2(ee)
                    dma_x(pf)
                do_mm1(bi)
                if bi > 0:
                    do_mm2(bi - 1)
            do_mm2(nb - 1)
    nc.finalize()
    return nc


def _route(inputs, Wg, bg, k):
    """Host gate: replicate reference numerics (fp32) for routing."""
    logits = inputs.astype(np.float32) @ Wg.astype(np.float32) + bg.astype(np.float32)
    sel = np.argsort(-logits, axis=1, kind="stable")[:, :k]  # == jax.lax.top_k order
    tl = np.take_along_axis(logits, sel, axis=1).astype(np.float32)
    m = tl.max(axis=1, keepdims=True)
    e = np.exp(tl - m, dtype=np.float32)
    w = (e / e.sum(axis=1, keepdims=True)).astype(np.float32)
    return sel, w


def _prepare(inputs, W1, b1, W2, idxs, wvals, counts, blocks):
    """Build the device input arrays (shared x/cw + per-core weight cuts)."""
    import ml_dtypes

    bf16 = ml_dtypes.bfloat16
    E = N_EXPERTS
    T = sum(counts)

    xg = np.empty((T, D_MODEL), dtype=bf16)
    expert_off = []
    off = 0
    for e in range(E):
        cnt = counts[e]
        expert_off.append(off)
        xg[off : off + cnt] = inputs[idxs[e]].astype(bf16)
        off += cnt

    xcols = np.empty((128, 8 * T), dtype=bf16)
    t0 = c0 = 0
    for e, blk in blocks:
        xb = xg[t0 : t0 + blk].reshape(blk, 8, 128).transpose(2, 1, 0)
        xcols[:, c0 : c0 + 8 * blk] = xb.reshape(128, 8 * blk)
        t0 += blk
        c0 += 8 * blk

    in_maps = []
    for core in range(N_CORES):
        c0f = core * FCUT
        # (e, d, p, ft, f) -> (p, e, ft, d, f)
        w1h = np.ascontiguousarray(
            W1[:, :, c0f : c0f + FCUT]
            .astype(bf16)
            .reshape(E, 8, 128, NFT, 128)
            .transpose(2, 0, 3, 1, 4)
            .reshape(128, WCOLS)
        )
        # (e, ft, p, d) -> (p, e, ft, d)
        w2h = np.ascontiguousarray(
            W2[:, c0f : c0f + FCUT, :]
            .astype(bf16)
            .reshape(E, NFT, 128, D_MODEL)
            .transpose(2, 0, 1, 3)
            .reshape(128, WCOLS)
        )
        # (e, ft, p) -> (p, e, ft)
        b1h = np.ascontiguousarray(
            b1[:, c0f : c0f + FCUT]
            .reshape(E, NFT, 128)
            .transpose(2, 0, 1)
            .reshape(128, E * NFT)
        ).astype(np.float32)
        in_maps.append({"w1": w1h, "w2": w2h, "b1": b1h, "x": xcols})
    return in_maps, expert_off


def kernel(inputs, Wg, bg, W1, b1, W2, b2, k):
    global LAST_EXEC_TIME_NS
    k = int(np.asarray(k))
    inputs = np.ascontiguousarray(np.asarray(inputs, dtype=np.float32))
    Wg = np.asarray(Wg, dtype=np.float32)
    bg = np.asarray(bg, dtype=np.float32)
    W1 = np.asarray(W1, dtype=np.float32)
    b1 = np.asarray(b1, dtype=np.float32)
    W2 = np.asarray(W2, dtype=np.float32)
    b2 = np.asarray(b2, dtype=np.float32)

    N, D = inputs.shape
    E = Wg.shape[1]
    assert E == N_EXPERTS and D == D_MODEL and W1.shape == (E, D, D_FF)

    sel, w = _route(inputs, Wg, bg, k)

    idxs, wvals = [], []
    for e in range(E):
        tok, slot = np.nonzero(sel == e)
        idxs.append(tok)
        wvals.append(w[tok, slot])
    counts = [len(ix) for ix in idxs]
    blocks = _blocks_from_tiles(counts)

    in_maps, expert_off = _prepare(inputs, W1, b1, W2, idxs, wvals, counts, blocks)

    key = tuple(counts)
    if key not in _NC_CACHE:
        _NC_CACHE[key] = _build_nc(counts)
    nc = _NC_CACHE[key]

    trace = bool(os.environ.get("BASS_TRACE"))
    res = None
    for attempt in range(3):
        try:
            res = run_bass_kernel_spmd(
                nc, in_maps, core_ids=list(range(N_CORES)), trace=trace
            )
            break
        except Exception:
            # transient NRT/device failures recover after a short pause
            if attempt == 2:
                raise
            import time

            time.sleep(20)
    LAST_EXEC_TIME_NS = getattr(res, "exec_time_ns", None)

    T = sum(counts)
    ysum = np.zeros((8, 128, T), dtype=np.float32)
    for c in range(N_CORES):
        ysum += np.asarray(res.results[c]["y"]).astype(np.float32)
    # d-major planes [8, 128, T] -> [T, 1024]
    yT = np.ascontiguousarray(ysum.reshape(D_MODEL, T).T)

    results = np.zeros((N, D), dtype=np.float32)
    for e in range(E):
        cnt = counts[e]
        o = expert_off[e]
        # device computed silu(x W1 + b1) @ W2 unscaled; apply the combine
        # weight and the b2 term here
        results[idxs[e]] += wvals[e][:, None] * (yT[o : o + cnt] + b2[e][None, :])
    return results.astype(np.float32)
